# revision 1
# baseline (speedup 1.0000x reference)
"""Trainium2 Bass kernel for nn_D_GCN (Chebyshev-style GCN diffusion).

Reference computation (per batch b):
    x0 = X                       (T, N, F) node features
    x1 = A x0                    (diffusion over nodes)
    x2 = 2 A x1 - x0
    out = relu(stack_k(x_k) @ Theta1 + bias)     Theta row index = f*K + k

Algebraic refactoring used here (Theta_k := Theta1[k::3]):
    out = relu( g0 + A @ (h1 + A @ h2) )
    g0  = x0 (Theta_0 - Theta_2) + bias    [host, f32]
    h1  = x0 Theta_1                       [host, f32]
    h2  = 2 x0 Theta_2                     [host, bf16]
All feature-dim matmuls (2% of FLOPs) are folded to host preprocessing;
the device runs the two dense N x N diffusion matmuls (98% of FLOPs,
all 128 MiB of A traffic) as plain chained matmuls with no transposes.

Sharding: 8 cores = 2 batches x 4 node-blocks of 1024 rows.
Each core holds A^T[:, block] (bf16, SBUF-resident across both passes),
computes y = A_blk @ h2 for its rows, w = h1 + y, AllGathers w within
its 4-core batch group, then out_blk = relu(A_blk @ w + g0).
"""

import sys

if "/opt/trn_rl_repo" not in sys.path:
    sys.path.insert(0, "/opt/trn_rl_repo")

import numpy as np
import ml_dtypes

B, T, N, F, O = 2, 8, 4096, 32, 32
K = 3
NCORES = 8
NB = 4            # node blocks per batch
RS = N // NB      # rows per shard = 1024
NC_CH = RS // 128  # 8 n-chunks per shard
KC = N // 128      # 32 k-chunks
TO = T * O         # 256 free columns

_CACHE = {}


def _build_nc():
    import concourse.mybir as mybir
    import concourse.tile as tile
    from concourse import bacc

    f32 = mybir.dt.float32
    bf16 = mybir.dt.bfloat16

    nc = bacc.Bacc(None, num_devices=NCORES)

    AT_d = nc.dram_tensor("AT", [KC, 128, RS], bf16, kind="ExternalInput")
    H2_d = nc.dram_tensor("H2", [KC, 128, TO], bf16, kind="ExternalInput")
    H1_d = nc.dram_tensor("H1", [NC_CH, 128, TO], f32, kind="ExternalInput")
    G0_d = nc.dram_tensor("G0", [NC_CH, 128, TO], f32, kind="ExternalInput")
    OUT_d = nc.dram_tensor("OUT", [NC_CH, 128, TO], f32, kind="ExternalOutput")

    RG = [[0, 1, 2, 3], [4, 5, 6, 7]]

    with tile.TileContext(nc) as tc:
        with (
            tc.tile_pool(name="big", bufs=1) as big,
            tc.tile_pool(name="ps", bufs=1, space="PSUM") as psp,
            tc.tile_pool(name="dram", bufs=1, space="DRAM") as dram,
        ):
            AT = big.tile([128, KC, RS], bf16, name="ATs", tag="ATs")
            H2 = big.tile([128, KC, TO], bf16, name="H2s", tag="H2s")
            W = big.tile([128, KC, TO], bf16, name="Ws", tag="Ws")
            H1 = big.tile([128, NC_CH, TO], f32, name="H1s", tag="H1s")
            G0 = big.tile([128, NC_CH, TO], f32, name="G0s", tag="G0s")
            WS = big.tile([128, NC_CH, TO], bf16, name="WSs", tag="WSs")
            OS = big.tile([128, NC_CH, TO], f32, name="OSs", tag="OSs")

            w_in = dram.tile([RS, TO], bf16, name="w_in", tag="w_in")
            w_all = dram.tile([N, TO], bf16, name="w_all", tag="w_all")

            # ---- input DMA streams (k-chunked for compute overlap) ----
            # H2 in 4 chunks of 8 k-chunks (512 KB); A in 8 chunks of 1 MiB.
            nc.sync.dma_start(
                H2[:, 0:8], H2_d[0:8].rearrange("k p n -> p k n"))
            nc.sync.dma_start(
                AT[:, 0:4], AT_d[0:4].rearrange("k p n -> p k n"))
            nc.sync.dma_start(
                H2[:, 8:16], H2_d[8:16].rearrange("k p n -> p k n"))
            nc.sync.dma_start(
                AT[:, 4:8], AT_d[4:8].rearrange("k p n -> p k n"))
            nc.sync.dma_start(
                H2[:, 16:24], H2_d[16:24].rearrange("k p n -> p k n"))
            nc.sync.dma_start(
                H2[:, 24:32], H2_d[24:32].rearrange("k p n -> p k n"))
            for c in range(2, 8):
                nc.sync.dma_start(
                    AT[:, c * 4:(c + 1) * 4],
                    AT_d[c * 4:(c + 1) * 4].rearrange("k p n -> p k n"))
            nc.scalar.dma_start(H1[:], H1_d[:].rearrange("k p n -> p k n"))
            nc.scalar.dma_start(G0[:], G0_d[:].rearrange("k p n -> p k n"))

            # ---- pass 1: y = A_blk @ h2, accumulated over k ----
            with nc.named_scope("pass1"):
                ps1 = [psp.tile([128, TO], f32, name=f"y{n}", tag=f"bank{n}")
                       for n in range(NC_CH)]
                for k in range(KC):
                    for n in range(NC_CH):
                        nc.tensor.matmul(
                            ps1[n][:],
                            AT[:, k, n * 128:(n + 1) * 128],
                            H2[:, k],
                            start=(k == 0),
                            stop=(k == KC - 1),
                        )

            # ---- w = h1 + y  (bf16), ship to AllGather ----
            with nc.named_scope("gather"):
                for n in range(NC_CH):
                    nc.vector.tensor_add(WS[:, n], ps1[n][:], H1[:, n])
                nc.sync.dma_start(
                    w_in[:].rearrange("(c p) n -> p c n", p=128), WS[:])
                nc.gpsimd.collective_compute(
                    "AllGather",
                    mybir.AluOpType.bypass,
                    replica_groups=RG,
                    ins=[w_in[:]],
                    outs=[w_all[:]],
                )
                w_all_k = w_all[:].rearrange("(k p) n -> k p n", p=128)
                for c in range(4):
                    nc.sync.dma_start(
                        W[:, c * 8:(c + 1) * 8],
                        w_all_k[c * 8:(c + 1) * 8].rearrange("k p n -> p k n"))

            # ---- pass 2: out = relu(A_blk @ w + g0) ----
            with nc.named_scope("pass2"):
                ps2 = [psp.tile([128, TO], f32, name=f"p{n}", tag=f"bank{n}")
                       for n in range(NC_CH)]
                for k in range(KC):
                    for n in range(NC_CH):
                        nc.tensor.matmul(
                            ps2[n][:],
                            AT[:, k, n * 128:(n + 1) * 128],
                            W[:, k],
                            start=(k == 0),
                            stop=(k == KC - 1),
                        )

            with nc.named_scope("epilogue"):
                Relu = mybir.ActivationFunctionType.Relu
                for n in range(NC_CH):
                    nc.vector.tensor_add(OS[:, n], ps2[n][:], G0[:, n])
                    nc.scalar.activation(OS[:, n], OS[:, n], Relu)
                half = NC_CH // 2
                nc.sync.dma_start(
                    OUT_d[0:half].rearrange("c p n -> p c n"), OS[:, 0:half])
                nc.sync.dma_start(
                    OUT_d[half:].rearrange("c p n -> p c n"), OS[:, half:])

    nc.compile()
    return nc


def _get_nc():
    if "nc" not in _CACHE:
        _CACHE["nc"] = _build_nc()
    return _CACHE["nc"]


def _prepare_in_maps(X, A_q, Theta1, bias):
    bf16 = ml_dtypes.bfloat16
    X = np.asarray(X, dtype=np.float32)
    A_q = np.asarray(A_q, dtype=np.float32)
    Theta1 = np.asarray(Theta1, dtype=np.float32)
    bias = np.asarray(bias, dtype=np.float32)

    Th = Theta1.reshape(F, K, O)
    Th0, Th1, Th2 = Th[:, 0], Th[:, 1], Th[:, 2]

    in_maps = []
    for b in range(B):
        Xb = X[b]                                   # (T, N, F)
        # [n, (t, o)] node-major layouts
        h2 = np.transpose(2.0 * (Xb @ Th2), (1, 0, 2)).reshape(N, TO)
        h1 = np.transpose(Xb @ Th1, (1, 0, 2)).reshape(N, TO)
        g0 = np.transpose(Xb @ (Th0 - Th2) + bias, (1, 0, 2)).reshape(N, TO)
        h2b = np.ascontiguousarray(h2).astype(bf16).reshape(KC, 128, TO)
        for j in range(NB):
            rows = slice(j * RS, (j + 1) * RS)
            AT = A_q[b, rows].T.astype(bf16).reshape(KC, 128, RS)
            in_maps.append({
                "AT": AT,
                "H2": h2b,
                "H1": np.ascontiguousarray(h1[rows]).reshape(NC_CH, 128, TO),
                "G0": np.ascontiguousarray(g0[rows]).reshape(NC_CH, 128, TO),
            })
    return in_maps


def run_with_results(inputs, **spmd_kwargs):
    """Returns (full_output, BassKernelResults). spmd_kwargs forwarded to
    run_bass_kernel_spmd (e.g. trace=True)."""
    from concourse.bass_utils import run_bass_kernel_spmd

    nc = _get_nc()
    in_maps = _prepare_in_maps(**inputs)
    res = run_bass_kernel_spmd(
        nc, in_maps, core_ids=list(range(NCORES)), **spmd_kwargs)

    out = np.empty((B, T, N, O), dtype=np.float32)
    for c in range(NCORES):
        b, j = divmod(c, NB)
        blk = res.results[c]["OUT"].reshape(RS, T, O)   # [n, t, o]
        out[b, :, j * RS:(j + 1) * RS, :] = np.transpose(blk, (1, 0, 2))
    return out, res


def kernel(X, A_q, Theta1, bias):
    out, _ = run_with_results(
        {"X": X, "A_q": A_q, "Theta1": Theta1, "bias": bias})
    return out
